# revision 11
# baseline (speedup 1.0000x reference)
"""Trainium2 Bass kernel for nn_EventTemplateBank (batched 1-D template-bank conv).

Math: score[b,t,e] = sum_{f,l} delayed[e,f,l] * x[b, t+40-l, f] / (L*F),
with delayed = delay-shifted templates (zero fill) and x zero-padded.

Device formulation (per core, data-parallel over batch):
  - Contract over a 128-position window on SBUF partitions.
  - Host pre-permutes x into overlapping-window scratch
        Xsc[b, f, k, n] = x[b, 48n + k - 39, f]   (k in [0,128), zero OOB)
    so every output t = 48n + D (D in [0,48)) has its full 80-tap window
    inside the k range of column n.
  - Toeplitz weights (host-built, tiny):
        W[s, f, k, 16d+e] = delayed[e, f, (8s+d) + 79 - k] / 480
    One PSUM tile per D-set s accumulates 6 matmuls (one per feature f):
        out[s][m=(d,e), n] += W[s,f].T @ Xsc[b,f]
  - Output written to DRAM in matmul-native layout; host re-permutes to (B,S,E).
"""

import numpy as np

import concourse.bass as bass  # noqa: F401  (AP types referenced in comments)
import concourse.mybir as mybir
from concourse import bacc
from concourse.bass_utils import run_bass_kernel_spmd
from concourse.tile import TileContext

# Problem shapes (hardcoded per contract)
B, S, F = 64, 32768, 6
E, L = 16, 80
MAX_DELAY = 10

NCORES = 8
BPC = B // NCORES          # batches per core
Q = 48                     # output positions per rhs column
KWIN = 128                 # contraction window (partitions)
NS = 6                     # D-sets of 8 -> D in [0, 48)
PADF = 39                  # window of column n starts at 48n - 39
NCOL = 688                 # columns per batch (ceil(32768/48)=683, padded to 2*344)
NBLK = 2
BLK = NCOL // NBLK         # 344 columns per matmul block

LAST_RESULT = None         # BassKernelResults of the most recent run (for profiling)


def _build_weights(templates: np.ndarray, onset_delays: np.ndarray) -> np.ndarray:
    """W[s, f, k, 16d+e] = delayed[e, f, (8s+d)+79-k] / (L*F), zero outside [0,L)."""
    d = np.round(np.clip(onset_delays, -MAX_DELAY, MAX_DELAY)).astype(np.int64)
    idx = np.arange(L)
    src = idx[None, None, :] - d[:, :, None]                 # (E,F,L)
    valid = (src >= 0) & (src < L)
    delayed = np.take_along_axis(templates, np.clip(src, 0, L - 1), axis=2)
    delayed = np.where(valid, delayed, 0.0).astype(np.float32) / float(L * F)

    # l_index[s, d, k] = 8s + d + 79 - k
    D = (8 * np.arange(NS)[:, None] + np.arange(8)[None, :])  # (NS, 8)
    l_idx = D[:, :, None] + 79 - np.arange(KWIN)[None, None, :]   # (NS, 8, K)
    ok = (l_idx >= 0) & (l_idx < L)
    l_cl = np.clip(l_idx, 0, L - 1)
    # gather: delayed[e, f, l_cl[s,dd,k]] -> (NS,8,K,E,F)
    g = delayed[:, :, l_cl]                                   # (E, F, NS, 8, K)
    g = np.where(ok[None, None], g, 0.0)
    # -> W[k, s, f, dd, e] -> (K, NS, F, 8*16)  (k-major so the DMA is contiguous)
    W = g.transpose(4, 2, 1, 3, 0).reshape(KWIN, NS, F, 128)
    return np.ascontiguousarray(W, dtype=np.float32)


def _build_xsc(x: np.ndarray) -> np.ndarray:
    """Xsc[b, k, f, n] = x[b, Q*n + k - PADF, f] with zero fill OOB."""
    need = Q * (NCOL - 1) + KWIN                      # last index touched + 1 (pre-pad frame)
    xpad = np.zeros((B, PADF + need, F), dtype=np.float32)
    xpad[:, PADF:PADF + S, :] = x
    sb, st, sf = xpad.strides
    v = np.lib.stride_tricks.as_strided(
        xpad, shape=(B, KWIN, F, NCOL), strides=(sb, st, sf, Q * st)
    )
    return np.ascontiguousarray(v)


def _build_program():
    f32 = mybir.dt.float32
    nc = bacc.Bacc("TRN2", target_bir_lowering=False, debug=False)
    xsc = nc.dram_tensor("xsc", [BPC, KWIN, F, NCOL], f32, kind="ExternalInput")
    w = nc.dram_tensor("w", [KWIN, NS, F, 128], f32, kind="ExternalInput")
    osc = nc.dram_tensor("osc", [BPC, NBLK, NS, 128, BLK], f32, kind="ExternalOutput")

    with TileContext(nc) as tc:
        # Every tile the PE consumes is staged through a DVE copy so each
        # matmul carries at most ONE semaphore wait (the fp32 self-loading
        # Matmult/S3_LW instruction only has a single sync-wait slot).
        with (
            tc.tile_pool(name="wp", bufs=1) as wp,
            tc.tile_pool(name="xp", bufs=2) as xp,
            tc.tile_pool(name="xsp", bufs=2) as xsp,
            tc.tile_pool(name="pp", bufs=8, space="PSUM") as pp,
            tc.tile_pool(name="op", bufs=4) as op,
        ):
            wt_raw = wp.tile([KWIN, NS * F * 128], f32)
            nc.sync.dma_start(out=wt_raw, in_=w.rearrange("k s f m -> k (s f m)"))
            wt = wp.tile([KWIN, NS * F * 128], f32)
            nc.vector.tensor_copy(out=wt, in_=wt_raw)
            for b in range(BPC):
                for blk in range(NBLK):
                    xt_raw = xp.tile([KWIN, F * BLK], f32)
                    nc.sync.dma_start(
                        out=xt_raw,
                        in_=xsc[b, :, :, blk * BLK:(blk + 1) * BLK],
                    )
                    xt = xsp.tile([KWIN, F * BLK], f32)
                    nc.vector.tensor_copy(out=xt, in_=xt_raw)
                    for s in range(NS):
                        ps = pp.tile([128, BLK], f32)
                        for f in range(F):
                            nc.tensor.matmul(
                                ps,
                                wt[:, (s * F + f) * 128:(s * F + f + 1) * 128],
                                xt[:, f * BLK:(f + 1) * BLK],
                                start=(f == 0),
                                stop=(f == F - 1),
                            )
                        ot = op.tile([128, BLK], f32)
                        nc.vector.tensor_copy(out=ot, in_=ps)
                        nc.sync.dma_start(out=osc[b, blk, s], in_=ot)
    nc.compile()   # bacc passes: split multi-waits (HW allows 1 wait/inst), DCE, reg alloc
    return nc


def kernel(x: np.ndarray, templates: np.ndarray, onset_delays: np.ndarray) -> np.ndarray:
    global LAST_RESULT
    x = np.ascontiguousarray(x, dtype=np.float32)
    templates = np.asarray(templates, dtype=np.float32)
    onset_delays = np.asarray(onset_delays, dtype=np.float32)

    W = _build_weights(templates, onset_delays)
    Xsc = _build_xsc(x)                                   # (B, F, K, NCOL)

    nc = _build_program()
    in_maps = [
        {"xsc": Xsc[c * BPC:(c + 1) * BPC], "w": W} for c in range(NCORES)
    ]
    res = run_bass_kernel_spmd(nc, in_maps, core_ids=list(range(NCORES)))
    LAST_RESULT = res

    osc = np.concatenate([r["osc"] for r in res.results], axis=0)  # (B,NBLK,NS,128,BLK)
    o = osc.reshape(B, NBLK, NS, 8, E, BLK)               # b, blk, s, d, e, n
    o = o.transpose(0, 1, 5, 2, 3, 4)                      # b, blk, n, s, d, e
    o = np.ascontiguousarray(o).reshape(B, NCOL * Q, E)[:, :S, :]
    o = np.ascontiguousarray(o)
    o[:, S - 1, :] = 0.0                                   # reference zero-pads last column
    return o


# revision 15
# speedup vs baseline: 2.1524x; 2.1524x over previous
"""Trainium2 Bass kernel for nn_EventTemplateBank (batched 1-D template-bank conv).

Math: score[b,t,e] = sum_{f,l} delayed[e,f,l] * x[b, t+40-l, f] / (L*F),
with delayed = delay-shifted templates (zero fill) and x zero-padded.

Device formulation (per core, data-parallel over batch):
  - Contract over a 128-position window on SBUF partitions.
  - Host pre-permutes x into overlapping-window scratch
        Xsc[b, f, k, n] = x[b, 48n + k - 39, f]   (k in [0,128), zero OOB)
    so every output t = 48n + D (D in [0,48)) has its full 80-tap window
    inside the k range of column n.
  - Toeplitz weights (host-built, tiny):
        W[s, f, k, 16d+e] = delayed[e, f, (8s+d) + 79 - k] / 480
    One PSUM tile per D-set s accumulates 6 matmuls (one per feature f):
        out[s][m=(d,e), n] += W[s,f].T @ Xsc[b,f]
  - Output written to DRAM in matmul-native layout; host re-permutes to (B,S,E).
"""

import numpy as np

import concourse.bass as bass  # noqa: F401  (AP types referenced in comments)
import concourse.mybir as mybir
from concourse import bacc
from concourse.bass_utils import run_bass_kernel_spmd
from concourse.tile import TileContext

# Problem shapes (hardcoded per contract)
B, S, F = 64, 32768, 6
E, L = 16, 80
MAX_DELAY = 10

NCORES = 8
BPC = B // NCORES          # batches per core
Q = 48                     # output positions per rhs column
KWIN = 128                 # contraction window (partitions)
NS = 6                     # D-sets of 8 -> D in [0, 48)
PADF = 39                  # window of column n starts at 48n - 39
NCOL = 688                 # columns per batch (ceil(32768/48)=683, padded to 2*344)
NBLK = 2
BLK = NCOL // NBLK         # 344 columns per matmul block

LAST_RESULT = None         # BassKernelResults of the most recent run (for profiling)


def _build_weights(templates: np.ndarray, onset_delays: np.ndarray) -> np.ndarray:
    """W[s, f, k, 16d+e] = delayed[e, f, (8s+d)+79-k] / (L*F), zero outside [0,L)."""
    d = np.round(np.clip(onset_delays, -MAX_DELAY, MAX_DELAY)).astype(np.int64)
    idx = np.arange(L)
    src = idx[None, None, :] - d[:, :, None]                 # (E,F,L)
    valid = (src >= 0) & (src < L)
    delayed = np.take_along_axis(templates, np.clip(src, 0, L - 1), axis=2)
    delayed = np.where(valid, delayed, 0.0).astype(np.float32) / float(L * F)

    # l_index[s, d, k] = 8s + d + 79 - k
    D = (8 * np.arange(NS)[:, None] + np.arange(8)[None, :])  # (NS, 8)
    l_idx = D[:, :, None] + 79 - np.arange(KWIN)[None, None, :]   # (NS, 8, K)
    ok = (l_idx >= 0) & (l_idx < L)
    l_cl = np.clip(l_idx, 0, L - 1)
    # gather: delayed[e, f, l_cl[s,dd,k]] -> (NS,8,K,E,F)
    g = delayed[:, :, l_cl]                                   # (E, F, NS, 8, K)
    g = np.where(ok[None, None], g, 0.0)
    # -> W[k, s, f, dd, e] -> (K, NS, F, 8*16)  (k-major so the DMA is contiguous)
    W = g.transpose(4, 2, 1, 3, 0).reshape(KWIN, NS, F, 128)
    return np.ascontiguousarray(W, dtype=np.float32)


def _build_xsc(x: np.ndarray) -> np.ndarray:
    """Xsc[b, k, f, n] = x[b, Q*n + k - PADF, f] with zero fill OOB."""
    need = Q * (NCOL - 1) + KWIN                      # last index touched + 1 (pre-pad frame)
    xpad = np.zeros((B, PADF + need, F), dtype=np.float32)
    xpad[:, PADF:PADF + S, :] = x
    sb, st, sf = xpad.strides
    v = np.lib.stride_tricks.as_strided(
        xpad, shape=(B, KWIN, F, NCOL), strides=(sb, st, sf, Q * st)
    )
    return np.ascontiguousarray(v)


def _build_program():
    f32 = mybir.dt.float32
    nc = bacc.Bacc("TRN2", target_bir_lowering=False, debug=False)
    xsc = nc.dram_tensor("xsc", [BPC, KWIN, F, NCOL], f32, kind="ExternalInput")
    w = nc.dram_tensor("w", [KWIN, NS, F, 128], f32, kind="ExternalInput")
    osc = nc.dram_tensor("osc", [BPC, NBLK, NS, 128, BLK], f32, kind="ExternalOutput")

    with TileContext(nc) as tc:
        # Every tile the PE consumes is staged through a DVE copy so each
        # matmul carries at most ONE semaphore wait (the fp32 self-loading
        # Matmult/S3_LW instruction only has a single sync-wait slot).
        with (
            tc.tile_pool(name="wp", bufs=1) as wp,
            tc.tile_pool(name="xp", bufs=2) as xp,
            tc.tile_pool(name="xsp", bufs=2) as xsp,
            tc.tile_pool(name="pp", bufs=8, space="PSUM") as pp,
            tc.tile_pool(name="op", bufs=4) as op,
        ):
            f32r = mybir.dt.float32r
            wt_raw = wp.tile([KWIN, NS * F * 128], f32)
            nc.sync.dma_start(out=wt_raw, in_=w.rearrange("k s f m -> k (s f m)"))
            wt = wp.tile([KWIN, NS * F * 128], f32r)
            nc.vector.tensor_copy(out=wt, in_=wt_raw)
            for b in range(BPC):
                for blk in range(NBLK):
                    xt_raw = xp.tile([KWIN, F * BLK], f32)
                    nc.sync.dma_start(
                        out=xt_raw,
                        in_=xsc[b, :, :, blk * BLK:(blk + 1) * BLK],
                    )
                    xt = xsp.tile([KWIN, F * BLK], f32r)
                    nc.vector.tensor_copy(out=xt, in_=xt_raw)
                    for s in range(NS):
                        ps = pp.tile([128, BLK], f32)
                        for f in range(F):
                            nc.tensor.matmul(
                                ps,
                                wt[:, (s * F + f) * 128:(s * F + f + 1) * 128],
                                xt[:, f * BLK:(f + 1) * BLK],
                                start=(f == 0),
                                stop=(f == F - 1),
                            )
                        ot = op.tile([128, BLK], f32)
                        nc.vector.tensor_copy(out=ot, in_=ps)
                        nc.sync.dma_start(out=osc[b, blk, s], in_=ot)
    nc.compile()   # bacc passes: split multi-waits (HW allows 1 wait/inst), DCE, reg alloc
    return nc


def kernel(x: np.ndarray, templates: np.ndarray, onset_delays: np.ndarray) -> np.ndarray:
    global LAST_RESULT
    x = np.ascontiguousarray(x, dtype=np.float32)
    templates = np.asarray(templates, dtype=np.float32)
    onset_delays = np.asarray(onset_delays, dtype=np.float32)

    W = _build_weights(templates, onset_delays)
    Xsc = _build_xsc(x)                                   # (B, F, K, NCOL)

    nc = _build_program()
    in_maps = [
        {"xsc": Xsc[c * BPC:(c + 1) * BPC], "w": W} for c in range(NCORES)
    ]
    res = run_bass_kernel_spmd(nc, in_maps, core_ids=list(range(NCORES)))
    LAST_RESULT = res

    osc = np.concatenate([r["osc"] for r in res.results], axis=0)  # (B,NBLK,NS,128,BLK)
    o = osc.reshape(B, NBLK, NS, 8, E, BLK)               # b, blk, s, d, e, n
    o = o.transpose(0, 1, 5, 2, 3, 4)                      # b, blk, n, s, d, e
    o = np.ascontiguousarray(o).reshape(B, NCOL * Q, E)[:, :S, :]
    o = np.ascontiguousarray(o)
    o[:, S - 1, :] = 0.0                                   # reference zero-pads last column
    return o


# revision 16
# speedup vs baseline: 3.0540x; 1.4188x over previous
"""Trainium2 Bass kernel for nn_EventTemplateBank (batched 1-D template-bank conv).

Math: score[b,t,e] = sum_{f,l} delayed[e,f,l] * x[b, t+40-l, f] / (L*F),
with delayed = delay-shifted templates (zero fill) and x zero-padded.

Device formulation (per core, data-parallel over batch):
  - Contract over a 128-position window on SBUF partitions.
  - Host pre-permutes x into overlapping-window scratch with one flat
    column axis across the core's 8 batches (683 columns per batch,
    zero-padded to 11*512):
        Xsc[k, f, c] = x[b, 48n + k - 39, f],  c = 683*b + n
    so every output t = 48n + D (D in [0,48)) has its full 80-tap window
    inside the k range of column c.
  - Toeplitz weights (host-built from the tiny templates):
        W[k, s, f, 16d+e] = delayed[e, f, (8s+d) + 79 - k] / 480
    One PSUM tile per D-set s accumulates 6 matmuls (one per feature f):
        out[s][m=(d,e), c-block] += W[:, s, f].T @ Xsc[:, f, c-block]
    Operands are float32r (single-pass PE, ~1 cycle/column at N=512).
  - Output written to DRAM in matmul-native layout; host re-permutes to (B,S,E).
"""

import numpy as np

import concourse.mybir as mybir
from concourse import bacc
from concourse.bass_utils import run_bass_kernel_spmd
from concourse.tile import TileContext

# Problem shapes (hardcoded per contract)
B, S, F = 64, 32768, 6
E, L = 16, 80
MAX_DELAY = 10

NCORES = 8
BPC = B // NCORES          # batches per core
Q = 48                     # output positions per rhs column
KWIN = 128                 # contraction window (partitions)
NS = 6                     # D-sets of 8 -> D in [0, 48)
PADF = 39                  # window of column n starts at 48n - 39
NCOLB = (S + Q - 1) // Q   # 683 columns per batch
BLKN = 512                 # columns per matmul block
NBLK = 11                  # ceil(8*683 / 512)
CPAD = NBLK * BLKN         # 5632 padded columns per core

LAST_RESULT = None         # BassKernelResults of the most recent run (for profiling)


def _build_weights(templates: np.ndarray, onset_delays: np.ndarray) -> np.ndarray:
    """W[k, s, f, 16d+e] = delayed[e, f, (8s+d)+79-k] / (L*F), zero outside [0,L)."""
    d = np.round(np.clip(onset_delays, -MAX_DELAY, MAX_DELAY)).astype(np.int64)
    idx = np.arange(L)
    src = idx[None, None, :] - d[:, :, None]                 # (E,F,L)
    valid = (src >= 0) & (src < L)
    delayed = np.take_along_axis(templates, np.clip(src, 0, L - 1), axis=2)
    delayed = np.where(valid, delayed, 0.0).astype(np.float32) / float(L * F)

    D = (8 * np.arange(NS)[:, None] + np.arange(8)[None, :])      # (NS, 8)
    l_idx = D[:, :, None] + 79 - np.arange(KWIN)[None, None, :]   # (NS, 8, K)
    ok = (l_idx >= 0) & (l_idx < L)
    g = delayed[:, :, np.clip(l_idx, 0, L - 1)]                   # (E, F, NS, 8, K)
    g = np.where(ok[None, None], g, 0.0)
    # -> W[k, s, f, dd, e] (k-major so the device DMA is contiguous)
    W = g.transpose(4, 2, 1, 3, 0).reshape(KWIN, NS, F, 128)
    return np.ascontiguousarray(W, dtype=np.float32)


def _build_xsc(x: np.ndarray) -> np.ndarray:
    """Xsc[core, k, f, c] = x[8*core + c//683, 48*(c%683) + k - 39, f], zero OOB/pad."""
    need = Q * (NCOLB - 1) + KWIN
    xpad = np.zeros((B, PADF + need, F), dtype=np.float32)
    xpad[:, PADF:PADF + S, :] = x
    sb, st, sf = xpad.strides
    v = np.lib.stride_tricks.as_strided(
        xpad, shape=(B, KWIN, F, NCOLB), strides=(sb, st, sf, Q * st)
    )
    out = np.zeros((NCORES, KWIN, F, CPAD), dtype=np.float32)
    for b in range(B):
        core, i = divmod(b, BPC)
        out[core, :, :, i * NCOLB:(i + 1) * NCOLB] = v[b]
    return out


def _build_program():
    f32 = mybir.dt.float32
    f32r = mybir.dt.float32r
    nc = bacc.Bacc("TRN2", target_bir_lowering=False, debug=False)
    xsc = nc.dram_tensor("xsc", [KWIN, F, CPAD], f32, kind="ExternalInput")
    w = nc.dram_tensor("w", [KWIN, NS, F, 128], f32, kind="ExternalInput")
    osc = nc.dram_tensor("osc", [NBLK, NS, 128, BLKN], f32, kind="ExternalOutput")

    with TileContext(nc) as tc:
        with (
            tc.tile_pool(name="wp", bufs=1) as wp,
            tc.tile_pool(name="xp", bufs=3) as xp,
            tc.tile_pool(name="pp", bufs=8, space="PSUM") as pp,
            tc.tile_pool(name="op", bufs=6) as op,
        ):
            # Weights: DMA then per-set DVE cast-copy to float32r.
            wt_raw = wp.tile([KWIN, NS * F * 128], f32)
            nc.sync.dma_start(out=wt_raw, in_=w.rearrange("k s f m -> k (s f m)"))
            wt = wp.tile([KWIN, NS * F * 128], f32r)
            for s in range(NS):
                nc.vector.tensor_copy(
                    out=wt[:, s * F * 128:(s + 1) * F * 128],
                    in_=wt_raw[:, s * F * 128:(s + 1) * F * 128],
                )
            for blk in range(NBLK):
                # SWDGE cast-DMA: f32 DRAM -> f32r SBUF (rounds during transfer)
                xt = xp.tile([KWIN, F * BLKN], f32r)
                nc.gpsimd.dma_start(
                    out=xt, in_=xsc[:, :, blk * BLKN:(blk + 1) * BLKN]
                )
                for s in range(NS):
                    ps = pp.tile([128, BLKN], f32)
                    for f in range(F):
                        nc.tensor.matmul(
                            ps,
                            wt[:, (s * F + f) * 128:(s * F + f + 1) * 128],
                            xt[:, f * BLKN:(f + 1) * BLKN],
                            start=(f == 0),
                            stop=(f == F - 1),
                        )
                    ot = op.tile([128, BLKN], f32)
                    nc.vector.tensor_copy(out=ot, in_=ps)
                    nc.sync.dma_start(out=osc[blk, s], in_=ot)
    nc.compile()   # bacc passes: split multi-waits (HW allows 1 wait/inst), DCE, reg alloc
    return nc


def kernel(x: np.ndarray, templates: np.ndarray, onset_delays: np.ndarray) -> np.ndarray:
    global LAST_RESULT
    x = np.ascontiguousarray(x, dtype=np.float32)
    templates = np.asarray(templates, dtype=np.float32)
    onset_delays = np.asarray(onset_delays, dtype=np.float32)

    W = _build_weights(templates, onset_delays)
    Xsc = _build_xsc(x)                                   # (NCORES, K, F, CPAD)

    nc = _build_program()
    in_maps = [{"xsc": Xsc[c], "w": W} for c in range(NCORES)]
    res = run_bass_kernel_spmd(nc, in_maps, core_ids=list(range(NCORES)))
    LAST_RESULT = res

    osc = np.stack([r["osc"] for r in res.results], axis=0)   # (NCORES,NBLK,NS,128,BLKN)
    o = osc.reshape(NCORES, NBLK, NS, 8, E, BLKN)             # core, blk, s, d, e, n
    o = o.transpose(0, 1, 5, 2, 3, 4)                          # core, blk, n, s, d, e
    o = np.ascontiguousarray(o).reshape(NCORES, CPAD, NS * 8 * E)
    o = o[:, :BPC * NCOLB, :].reshape(NCORES, BPC, NCOLB, NS, 8, E)
    o = o.reshape(B, NCOLB * Q, E)[:, :S, :]
    o = np.ascontiguousarray(o)
    o[:, S - 1, :] = 0.0                                   # reference zero-pads last column
    return o


# revision 17
# speedup vs baseline: 3.0561x; 1.0007x over previous
"""Trainium2 Bass kernel for nn_EventTemplateBank (batched 1-D template-bank conv).

Math: score[b,t,e] = sum_{f,l} delayed[e,f,l] * x[b, t+40-l, f] / (L*F),
with delayed = delay-shifted templates (zero fill) and x zero-padded.

Device formulation (per core, data-parallel over batch):
  - Contract over a 128-position window on SBUF partitions.
  - Host pre-permutes x into overlapping-window scratch with one flat
    column axis across the core's 8 batches (683 columns per batch,
    zero-padded to 11*512):
        Xsc[k, f, c] = x[b, 48n + k - 39, f],  c = 683*b + n
    so every output t = 48n + D (D in [0,48)) has its full 80-tap window
    inside the k range of column c.
  - Toeplitz weights (host-built from the tiny templates):
        W[k, s, f, 16d+e] = delayed[e, f, (8s+d) + 79 - k] / 480
    One PSUM tile per D-set s accumulates 6 matmuls (one per feature f):
        out[s][m=(d,e), c-block] += W[:, s, f].T @ Xsc[:, f, c-block]
    Operands are float32r (single-pass PE, ~1 cycle/column at N=512).
  - Output written to DRAM in matmul-native layout; host re-permutes to (B,S,E).
"""

import numpy as np

import concourse.mybir as mybir
from concourse import bacc
from concourse.bass_utils import run_bass_kernel_spmd
from concourse.tile import TileContext

# Problem shapes (hardcoded per contract)
B, S, F = 64, 32768, 6
E, L = 16, 80
MAX_DELAY = 10

NCORES = 8
BPC = B // NCORES          # batches per core
Q = 48                     # output positions per rhs column
KWIN = 128                 # contraction window (partitions)
NS = 6                     # D-sets of 8 -> D in [0, 48)
PADF = 39                  # window of column n starts at 48n - 39
NCOLB = (S + Q - 1) // Q   # 683 columns per batch
BLKN = 512                 # columns per matmul block
NBLK = 11                  # ceil(8*683 / 512)
CPAD = NBLK * BLKN         # 5632 padded columns per core

LAST_RESULT = None         # BassKernelResults of the most recent run (for profiling)


def _build_weights(templates: np.ndarray, onset_delays: np.ndarray) -> np.ndarray:
    """W[k, s, f, 16d+e] = delayed[e, f, (8s+d)+79-k] / (L*F), zero outside [0,L)."""
    d = np.round(np.clip(onset_delays, -MAX_DELAY, MAX_DELAY)).astype(np.int64)
    idx = np.arange(L)
    src = idx[None, None, :] - d[:, :, None]                 # (E,F,L)
    valid = (src >= 0) & (src < L)
    delayed = np.take_along_axis(templates, np.clip(src, 0, L - 1), axis=2)
    delayed = np.where(valid, delayed, 0.0).astype(np.float32) / float(L * F)

    D = (8 * np.arange(NS)[:, None] + np.arange(8)[None, :])      # (NS, 8)
    l_idx = D[:, :, None] + 79 - np.arange(KWIN)[None, None, :]   # (NS, 8, K)
    ok = (l_idx >= 0) & (l_idx < L)
    g = delayed[:, :, np.clip(l_idx, 0, L - 1)]                   # (E, F, NS, 8, K)
    g = np.where(ok[None, None], g, 0.0)
    # -> W[k, s, f, dd, e] (k-major so the device DMA is contiguous)
    W = g.transpose(4, 2, 1, 3, 0).reshape(KWIN, NS, F, 128)
    return np.ascontiguousarray(W, dtype=np.float32)


def _build_xsc(x: np.ndarray) -> np.ndarray:
    """Xsc[core, k, f, c] = x[8*core + c//683, 48*(c%683) + k - 39, f], zero OOB/pad."""
    need = Q * (NCOLB - 1) + KWIN
    xpad = np.zeros((B, PADF + need, F), dtype=np.float32)
    xpad[:, PADF:PADF + S, :] = x
    sb, st, sf = xpad.strides
    v = np.lib.stride_tricks.as_strided(
        xpad, shape=(B, KWIN, F, NCOLB), strides=(sb, st, sf, Q * st)
    )
    out = np.zeros((NCORES, KWIN, F, CPAD), dtype=np.float32)
    for b in range(B):
        core, i = divmod(b, BPC)
        out[core, :, :, i * NCOLB:(i + 1) * NCOLB] = v[b]
    return out


def _build_program():
    f32 = mybir.dt.float32
    f32r = mybir.dt.float32r
    nc = bacc.Bacc("TRN2", target_bir_lowering=False, debug=False)
    xsc = nc.dram_tensor("xsc", [KWIN, F, CPAD], f32, kind="ExternalInput")
    w = nc.dram_tensor("w", [KWIN, NS, F, 128], f32, kind="ExternalInput")
    osc = nc.dram_tensor("osc", [NBLK, NS, 128, BLKN], f32, kind="ExternalOutput")

    with TileContext(nc) as tc:
        with (
            tc.tile_pool(name="wp", bufs=1) as wp,
            tc.tile_pool(name="xp", bufs=3) as xp,
            tc.tile_pool(name="pp", bufs=8, space="PSUM") as pp,
            tc.tile_pool(name="op", bufs=6) as op,
        ):
            # First x block queued before the weights so both transfer at t=0
            # and the s=0 weights (smallest piece) gate the first matmul.
            xt0 = xp.tile([KWIN, F * BLKN], f32r, tag="xt")
            nc.gpsimd.dma_start(out=xt0, in_=xsc[:, :, 0:BLKN])
            # Weights: per-set DMA + DVE cast-copy to float32r.
            wt_raw = wp.tile([KWIN, NS * F * 128], f32)
            wt = wp.tile([KWIN, NS * F * 128], f32r)
            wr = w.rearrange("k s f m -> k (s f m)")
            for s in range(NS):
                sl = slice(s * F * 128, (s + 1) * F * 128)
                nc.sync.dma_start(out=wt_raw[:, sl], in_=wr[:, sl])
                nc.vector.tensor_copy(out=wt[:, sl], in_=wt_raw[:, sl])
            for blk in range(NBLK):
                if blk == 0:
                    xt = xt0
                else:
                    # SWDGE cast-DMA: f32 DRAM -> f32r SBUF (rounds in transfer)
                    xt = xp.tile([KWIN, F * BLKN], f32r, tag="xt")
                    nc.gpsimd.dma_start(
                        out=xt, in_=xsc[:, :, blk * BLKN:(blk + 1) * BLKN]
                    )
                for s in range(NS):
                    ps = pp.tile([128, BLKN], f32)
                    for f in range(F):
                        nc.tensor.matmul(
                            ps,
                            wt[:, (s * F + f) * 128:(s * F + f + 1) * 128],
                            xt[:, f * BLKN:(f + 1) * BLKN],
                            start=(f == 0),
                            stop=(f == F - 1),
                        )
                    ot = op.tile([128, BLKN], f32)
                    nc.vector.tensor_copy(out=ot, in_=ps)
                    nc.sync.dma_start(out=osc[blk, s], in_=ot)
    nc.compile()   # bacc passes: split multi-waits (HW allows 1 wait/inst), DCE, reg alloc
    return nc


def kernel(x: np.ndarray, templates: np.ndarray, onset_delays: np.ndarray) -> np.ndarray:
    global LAST_RESULT
    x = np.ascontiguousarray(x, dtype=np.float32)
    templates = np.asarray(templates, dtype=np.float32)
    onset_delays = np.asarray(onset_delays, dtype=np.float32)

    W = _build_weights(templates, onset_delays)
    Xsc = _build_xsc(x)                                   # (NCORES, K, F, CPAD)

    nc = _build_program()
    in_maps = [{"xsc": Xsc[c], "w": W} for c in range(NCORES)]
    res = run_bass_kernel_spmd(nc, in_maps, core_ids=list(range(NCORES)))
    LAST_RESULT = res

    osc = np.stack([r["osc"] for r in res.results], axis=0)   # (NCORES,NBLK,NS,128,BLKN)
    o = osc.reshape(NCORES, NBLK, NS, 8, E, BLKN)             # core, blk, s, d, e, n
    o = o.transpose(0, 1, 5, 2, 3, 4)                          # core, blk, n, s, d, e
    o = np.ascontiguousarray(o).reshape(NCORES, CPAD, NS * 8 * E)
    o = o[:, :BPC * NCOLB, :].reshape(NCORES, BPC, NCOLB, NS, 8, E)
    o = o.reshape(B, NCOLB * Q, E)[:, :S, :]
    o = np.ascontiguousarray(o)
    o[:, S - 1, :] = 0.0                                   # reference zero-pads last column
    return o
